# revision 35
# baseline (speedup 1.0000x reference)
"""Trainium2 Bass kernel for nn_DecoderRNN (LSTM decoder + vocab projection).

Strategy (8 NeuronCores):
  - Vocab-shard the output projection 8-way (fc_W.T columns); replicate the
    LSTM recurrence on every core (the W_hh stream through the PE is
    batch-independent, so batch-sharding the recurrence buys nothing).
  - Batch-major gate layout [B=64, 4H] so the elementwise step runs as a few
    large ACT/DVE ops; h is transposed per step via the PE into a K-major
    h_seqT [H, tokens] buffer that feeds both the next step's matmul
    (stationary operand) and the final vocab projection.
  - Embedding rows are gathered on-device with indirect DMA, transposed via
    the PE into xsT [E, tokens] (bf16); each step's gate PSUM accumulates
    x-part (bf16 matmuls) + h-part (f32r matmuls) in one accumulation group.
  - Output logits are produced token-major [T*B, V_local] and re-assembled on
    the host (transpose + concat over vocab shards).
"""

import numpy as np
import ml_dtypes

import concourse.bacc as bacc
import concourse.mybir as mybir
import concourse.tile as tile
from concourse.bass import IndirectOffsetOnAxis
from concourse.bass_utils import run_bass_kernel_spmd
from concourse.masks import make_identity

B, T, E, H, V = 64, 32, 512, 512, 10000
G4 = 4 * H            # 2048 gate dims
NTOK = B * T          # 2048 tokens
NCORES = 8
VL = V // NCORES      # 1250 vocab per core
VLP = 1280            # padded so every fp32r matmul has N >= 256
KE = E // 128         # 4 K-chunks over E
KH = H // 128         # 4 K-chunks over H
MTOK = NTOK // 128    # 16 token chunks
MLOC = MTOK            # x_proj chunks computed locally (replicated)
OOB = 1 << 30          # gather index sentinel: skipped via bounds_check

F32 = mybir.dt.float32
F32R = mybir.dt.float32r
BF16 = mybir.dt.bfloat16
I32 = mybir.dt.int32

AFT = mybir.ActivationFunctionType

# proj N-chunks: (offset, size) within VLP
PROJ_CHUNKS = [(0, 512), (512, 512), (1024, 256)]


def _drain(nc, i, out_ap, in_ap):
    """PSUM->SBUF copy, alternating engines to split the drain load."""
    if i % 2 == 0:
        nc.vector.tensor_copy(out_ap, in_ap)
    else:
        nc.scalar.copy(out_ap, in_ap)


def build_nc(with_gate_bias: bool, with_fc_bias: bool):
    nc = bacc.Bacc("TRN2", target_bir_lowering=False, debug=False, num_devices=NCORES)

    emb_d = nc.dram_tensor("emb", [V, E], F32, kind="ExternalInput")
    idx_d = nc.dram_tensor("idx", [MLOC * 128, 1], I32, kind="ExternalInput")
    featT_d = nc.dram_tensor("featT", [B, E], F32, kind="ExternalInput")
    wxT_d = nc.dram_tensor("wxT", [KE, 128, G4], BF16, kind="ExternalInput")
    whT_d = nc.dram_tensor("whT", [KH, 128, G4], F32R, kind="ExternalInput")
    fcT_d = nc.dram_tensor("fcT", [KH, 128, VLP], F32R, kind="ExternalInput")
    bih_d = nc.dram_tensor("bih", [1, G4], F32, kind="ExternalInput")
    bhh_d = nc.dram_tensor("bhh", [1, G4], F32, kind="ExternalInput")
    fcb_d = nc.dram_tensor("fcb", [1, VLP], F32R, kind="ExternalInput")
    out_d = nc.dram_tensor("out", [NTOK, VL], F32, kind="ExternalOutput")

    with tile.TileContext(nc) as tc:
        build_body(
            nc, tc,
            emb_d, idx_d, featT_d, wxT_d, whT_d, fcT_d, bih_d, bhh_d, fcb_d,
            out_d, with_gate_bias, with_fc_bias,
        )
    nc.compile()
    return nc


def build_body(nc, tc, emb_d, idx_d, featT_d, wxT_d, whT_d, fcT_d, bih_d,
               bhh_d, fcb_d, out_d, with_gate_bias, with_fc_bias):
    from contextlib import ExitStack

    ctx = ExitStack()
    with ctx:
        const = ctx.enter_context(tc.tile_pool(name="const", bufs=1))
        gpool = ctx.enter_context(tc.tile_pool(name="gather", bufs=3))
        ipool = ctx.enter_context(tc.tile_pool(name="idx", bufs=3))
        tpsum = ctx.enter_context(tc.tile_pool(name="tpsum", bufs=2, space="PSUM"))
        gatesp = ctx.enter_context(tc.tile_pool(name="gates", bufs=1, space="PSUM"))
        projp = ctx.enter_context(tc.tile_pool(name="projp", bufs=2, space="PSUM"))
        ew = ctx.enter_context(tc.tile_pool(name="ew", bufs=3))
        state = ctx.enter_context(tc.tile_pool(name="state", bufs=1))
        stage = ctx.enter_context(tc.tile_pool(name="stage", bufs=3))

        # ---- constants / weights ----
        ident = const.tile([128, 128], F32)
        make_identity(nc, ident[:])
        identb = const.tile([128, 128], BF16)
        make_identity(nc, identb[:])

        wx_sb = const.tile([128, KE * G4], BF16)
        for k in range(KE):
            nc.sync.dma_start(wx_sb[:, k * G4:(k + 1) * G4], wxT_d[k])
        wh_sb = const.tile([128, KH * G4], F32R)
        for k in range(KH):
            nc.sync.dma_start(wh_sb[:, k * G4:(k + 1) * G4], whT_d[k])
        fc_sb = const.tile([128, KH * VLP], F32R)
        for k in range(KH):
            nc.sync.dma_start(fc_sb[:, k * VLP:(k + 1) * VLP], fcT_d[k])

        if with_gate_bias:
            brow_ih = const.tile([1, G4], F32)
            nc.sync.dma_start(brow_ih[:], bih_d.ap())
            brow_hh = const.tile([1, G4], F32)
            nc.sync.dma_start(brow_hh[:], bhh_d.ap())
            bias_g = const.tile([1, G4], F32R)
            nc.vector.tensor_add(bias_g[:], brow_ih[:], brow_hh[:])
            ones_row = const.tile([1, B], F32R)
            nc.gpsimd.memset(ones_row[:], 1.0)
        if with_fc_bias:
            fcb_sb = const.tile([1, VLP], F32R)
            nc.sync.dma_start(fcb_sb[:], fcb_d.ap())
            ones_row128 = const.tile([1, 128], F32R)
            nc.gpsimd.memset(ones_row128[:], 1.0)

        # ---- gather embeddings and transpose into xsT [E, tokens] ----
        # xsT holds xs.T, bf16 (E-chunk k at free offset k*NTOK)
        xsT = const.tile([128, KE * NTOK], BF16)
        for m in range(MLOC):
            it = ipool.tile([128, 1], I32)
            nc.sync.dma_start(it[:], idx_d[m * 128:(m + 1) * 128, :])
            xs = gpool.tile([128, E], F32)
            # chunk 0, rows 0..63 are step t=0: the image features. Their idx
            # entries are OOB sentinels, so the indirect gather skips those
            # rows and the pre-written features survive.
            if m == 0:
                nc.sync.dma_start(xs[0:B, :], featT_d.ap())
            nc.gpsimd.indirect_dma_start(
                out=xs[:], out_offset=None, in_=emb_d.ap(),
                in_offset=IndirectOffsetOnAxis(ap=it[:, :1], axis=0),
                bounds_check=V - 1, oob_is_err=False,
            )
            tp = tpsum.tile([128, 512], F32, name="tp_xs", tag="tp")
            for k in range(KE):
                nc.tensor.transpose(
                    tp[:, k * 128:(k + 1) * 128], xs[:, k * 128:(k + 1) * 128],
                    ident[:],
                )
            dst = xsT[:].rearrange("p (k t) -> p k t", k=KE)[:, :, m * 128:(m + 1) * 128]
            src = tp[:].rearrange("p (k t) -> p k t", k=KE)
            _drain(nc, m, dst, src)

        # ---- LSTM scan ----
        h_seqT = const.tile([128, KH * NTOK], F32R)  # h.T, chunk k at k*NTOK
        c_t = state.tile([B, H], F32)


        proj_stage = {}

        def emit_proj_group(nc, m, j):
            # one PSUM-bank group of the vocab projection for token chunk m
            if m not in proj_stage:
                proj_stage[m] = stage.tile([128, VLP], F32, name="st", tag="st")
            st = proj_stage[m]
            n0, nsz = PROJ_CHUNKS[j]
            pj = projp.tile([128, 512], F32, name="pj", tag="pj")
            for k in range(KH):
                nc.tensor.matmul(
                    pj[:, :nsz],
                    lhsT=h_seqT[:, k * NTOK + m * 128:
                                k * NTOK + (m + 1) * 128],
                    rhs=fc_sb[:, k * VLP + n0:k * VLP + n0 + nsz],
                    start=(k == 0),
                    stop=(k == KH - 1) and not with_fc_bias,
                    skip_group_check=True,
                )
            if with_fc_bias:
                nc.tensor.matmul(
                    pj[:, :nsz],
                    lhsT=ones_row128[:],
                    rhs=fcb_sb[:, n0:n0 + nsz],
                    start=False, stop=True, skip_group_check=True,
                )
            _drain(nc, m + j, st[:, n0:n0 + nsz], pj[:, :nsz])
            if j == len(PROJ_CHUNKS) - 1:
                nc.sync.dma_start(out_d[m * 128:(m + 1) * 128, :], st[:, :VL])
                del proj_stage[m]

        proj_emitted = 0
        NGROUPS = MTOK * len(PROJ_CHUNKS)

        def pace_proj(nc, t):
            # after step t, chunks 0..(t-2)//2 have both steps in h_seqT
            nonlocal proj_emitted
            if t < 3:
                return
            allowed = len(PROJ_CHUNKS) * ((t - 2) // 2 + 1)
            paced = min(allowed, NGROUPS * (t - 2) // (T - 4))
            while proj_emitted < paced:
                m, j = divmod(proj_emitted, len(PROJ_CHUNKS))
                emit_proj_group(nc, m, j)
                proj_emitted += 1

        for t in range(T):
            gates = [gatesp.tile([B, 512], F32, name=f"gates{n}", tag=f"gates{n}") for n in range(4)]
            for n in (1, 0, 2, 3):  # f first: sigmoid(f) gates the c-chain
                g = gates[n][:]
                mms = []
                # x-part (bf16): stationary = xsT token slice, moving = W_ih.T
                for k in range(KE):
                    mms.append((
                        xsT[:, k * NTOK + t * B:k * NTOK + (t + 1) * B],
                        wx_sb[:, k * G4 + n * 512:k * G4 + (n + 1) * 512],
                    ))
                if with_gate_bias:
                    mms.append((
                        ones_row[:],
                        bias_g[:, n * 512:(n + 1) * 512],
                    ))
                if t > 0:
                    # h-part (f32r): stationary = h.T of prev step
                    for k in range(KH):
                        mms.append((
                            h_seqT[:, k * NTOK + (t - 1) * B:
                                   k * NTOK + t * B],
                            wh_sb[:, k * G4 + n * 512:
                                  k * G4 + (n + 1) * 512],
                        ))
                for q, (lh, rh) in enumerate(mms):
                    nc.tensor.matmul(
                        g, lhsT=lh, rhs=rh,
                        start=(q == 0), stop=(q == len(mms) - 1),
                        skip_group_check=True,
                    )

            f_s = ew.tile([B, 512], BF16, name="f_s", tag="f_s")
            nc.scalar.activation(f_s[:], gates[1][:], AFT.Sigmoid)
            i_s = ew.tile([B, 512], BF16, name="i_s", tag="i_s")
            nc.scalar.activation(i_s[:], gates[0][:], AFT.Sigmoid)
            g_t = ew.tile([B, 512], BF16, name="g_t", tag="g_t")
            nc.scalar.activation(g_t[:], gates[2][:], AFT.Tanh)
            o_s = ew.tile([B, 512], BF16, name="o_s", tag="o_s")
            nc.scalar.activation(o_s[:], gates[3][:], AFT.Sigmoid)

            u = ew.tile([B, 512], F32, name="u", tag="u")
            if t == 0:
                nc.vector.tensor_mul(u[:], i_s[:], g_t[:])
                nc.vector.tensor_copy(c_t[:], u[:])
            else:
                fc_ = ew.tile([B, 512], F32, name="fc_", tag="fc_")
                nc.vector.tensor_mul(fc_[:], f_s[:], c_t[:])
                nc.vector.tensor_mul(u[:], i_s[:], g_t[:])
                nc.vector.tensor_add(c_t[:], fc_[:], u[:])
            tc_t = ew.tile([B, 512], BF16, name="tc_t", tag="tc_t")
            nc.scalar.activation(tc_t[:], c_t[:], AFT.Tanh)
            h_t = ew.tile([B, 512], BF16, name="h_t", tag="h_t")
            nc.vector.tensor_mul(h_t[:], o_s[:], tc_t[:])

            # transpose h_t into h_seqT columns for step t
            tp = tpsum.tile([128, 512], BF16, name="tp_h", tag="tp")
            for k in range(KH):
                nc.tensor.transpose(
                    tp[:, k * B:(k + 1) * B], h_t[:, k * 128:(k + 1) * 128],
                    identb[0:B, 0:B],
                )
            dst = h_seqT[:].rearrange("p (k t) -> p k t", k=KH)[:, :, t * B:(t + 1) * B]
            src = tp[:, 0:KH * B].rearrange("p (k t) -> p k t", k=KH)
            nc.vector.tensor_copy(dst, src)

            pace_proj(nc, t)


        # ---- remaining vocab-projection groups ----
        while proj_emitted < NGROUPS:
            m, j = divmod(proj_emitted, len(PROJ_CHUNKS))
            emit_proj_group(nc, m, j)
            proj_emitted += 1


_CACHE = {}


def _get_nc(with_gate_bias, with_fc_bias):
    key = (with_gate_bias, with_fc_bias)
    if key not in _CACHE:
        _CACHE[key] = build_nc(with_gate_bias, with_fc_bias)
    return _CACHE[key]


LAST_RESULTS = None


def kernel(features, captions, embed_W, W_ih, W_hh, b_ih, b_hh, fc_W, fc_b,
           _trace=False):
    global LAST_RESULTS
    features = np.asarray(features, dtype=np.float32)
    captions = np.asarray(captions)
    embed_W = np.asarray(embed_W, dtype=np.float32)
    W_ih = np.asarray(W_ih, dtype=np.float32)
    W_hh = np.asarray(W_hh, dtype=np.float32)
    b_ih = np.asarray(b_ih, dtype=np.float32)
    b_hh = np.asarray(b_hh, dtype=np.float32)
    fc_W = np.asarray(fc_W, dtype=np.float32)
    fc_b = np.asarray(fc_b, dtype=np.float32)

    with_gate_bias = bool(np.any(b_ih) or np.any(b_hh))
    with_fc_bias = bool(np.any(fc_b))
    nc = _get_nc(with_gate_bias, with_fc_bias)

    # token-major caption indices; t=0 slots hold OOB sentinels (features)
    idx = np.zeros((T, B), np.int32)
    idx[1:] = captions.astype(np.int64).T[1:].astype(np.int32)
    idx[0] = OOB
    idx = np.ascontiguousarray(idx.reshape(NTOK))

    wxT = np.ascontiguousarray(W_ih.T).astype(ml_dtypes.bfloat16).reshape(KE, 128, G4)
    whT = np.ascontiguousarray(W_hh.T).reshape(KH, 128, G4)
    fcT_full = np.ascontiguousarray(fc_W.T)  # [H, V]

    in_maps = []
    for c in range(NCORES):
        fcT_c = np.zeros((H, VLP), np.float32)
        fcT_c[:, :VL] = fcT_full[:, c * VL:(c + 1) * VL]
        fcb_c = np.zeros((1, VLP), np.float32)
        fcb_c[0, :VL] = fc_b[c * VL:(c + 1) * VL]
        in_maps.append({
            "emb": embed_W,
            "idx": np.ascontiguousarray(idx.reshape(-1, 1)),
            "featT": features,
            "wxT": wxT,
            "whT": whT,
            "fcT": np.ascontiguousarray(fcT_c.reshape(KH, 128, VLP)),
            "bih": b_ih.reshape(1, G4),
            "bhh": b_hh.reshape(1, G4),
            "fcb": fcb_c,
        })

    try:
        res = run_bass_kernel_spmd(nc, in_maps, list(range(NCORES)), trace=_trace)
    except ModuleNotFoundError:
        # no NTFF profile hook in this environment; run without tracing
        res = run_bass_kernel_spmd(nc, in_maps, list(range(NCORES)))
    LAST_RESULTS = res

    outs = [
        res.results[c]["out"].reshape(T, B, VL).transpose(1, 0, 2)
        for c in range(NCORES)
    ]
    return np.ascontiguousarray(np.concatenate(outs, axis=2), dtype=np.float32)

